# revision 28
# baseline (speedup 1.0000x reference)
"""Block-sparse attention (SageAttention-style mean-similarity top-k) on 8 TRN2 NeuronCores.

Sharding: 16 heads tensor-parallel across 8 cores (2 heads/core).
  - qkv weight column-sharded per core (its 2 heads' q/k/v rows, pre-transposed on host)
  - block selection + block-sparse attention fully local per head
  - proj weight row-sharded: each core computes the full-shape PARTIAL product
    o_local @ projW[:, c_slice].T (+ bias on core 0 only); the host unshard step
    sums the 8 partials (the row-parallel reduction).

Per-core device pipeline (bf16 compute, f32 selection):
  x^T f32 -> block sums (DVE) -> qm/km/sim (f32 PE) -> top-16 via max8/max_index
  qkv matmuls (bf16 PE), k kept d-major, v token-major with a ones column
  per query block: ap_gather (GPSIMD ucode, SBUF->SBUF) pulls the 16 selected
  k/v blocks; scores s^T = k_sel^T q (two heads packed in the 128x128 PE array
  via row groups); exp on ACT straight from PSUM; o = (e^T)^T v_sel with the
  gathered ones column yielding the softmax denominator; per-partition
  normalize; chunk-wise DMA-transpose + projection partials streamed out.
"""

import os
import sys

for _p in ("/opt/trn_rl_repo", "/root/.axon_site/_ro/trn_rl_repo"):
    if os.path.isdir(_p) and _p not in sys.path:
        sys.path.insert(0, _p)

import numpy as np

import concourse.bass as bass
import concourse.bacc as bacc
import concourse.tile as tile
import concourse.mybir as mybir
from concourse.bass_utils import run_bass_kernel_spmd
from concourse.library_config import ap_gather as ap_gather_lib

# problem constants
N = 4096          # sequence length
C = 1024          # model dim
H = 16            # heads
D = 64            # head dim
BLK = 128         # block size
NB = N // BLK     # 32 blocks
TOPK = 16         # int(0.5 * NB)
NCORES = 8
HPC = H // NCORES  # 2 heads per core
SCALE = D ** -0.5  # 0.125

F32 = mybir.dt.float32
BF16 = mybir.dt.bfloat16
I16 = mybir.dt.int16
U32 = mybir.dt.uint32

_CACHE = {}


def _build():
    nc = bacc.Bacc("TRN2", target_bir_lowering=False, debug=False,
                   num_devices=NCORES)

    KC = C // 128  # 8 contraction tiles

    xT = nc.dram_tensor("xT", [C, N], F32, kind="ExternalInput")
    wqkvT = nc.dram_tensor("wqkvT", [C, 3 * 2 * D], F32, kind="ExternalInput")
    projWT = nc.dram_tensor("projWT", [2 * D, C], F32, kind="ExternalInput")
    projb = nc.dram_tensor("projb", [128, KC], F32, kind="ExternalInput")
    ident64 = nc.dram_tensor("ident64", [64, 64], F32, kind="ExternalInput")
    erep = nc.dram_tensor("erep", [16, 128], F32, kind="ExternalInput")
    out_ext = nc.dram_tensor("out", [C, N], F32, kind="ExternalOutput")

    obounce = nc.dram_tensor("obounce", [N, 2 * D], BF16)

    with tile.TileContext(nc) as tc:
        nc.gpsimd.load_library(ap_gather_lib)

        with tc.tile_pool(name="persist", bufs=1) as pp:
            # ---- weights ----
            wqkv_bf = pp.tile([128, KC, 384], BF16)
            nc.gpsimd.dma_start(
                wqkv_bf[:], wqkvT.ap().rearrange("(a p) m -> p a m", p=128))
            wqk_f32 = pp.tile([128, KC, 256], F32)
            nc.sync.dma_start(
                wqk_f32[:], wqkvT.ap().rearrange("(a p) m -> p a m", p=128)[:, :, 0:256])
            projW_bf = pp.tile([128, C], BF16)          # [c_local, j]
            nc.gpsimd.dma_start(projW_bf[:], projWT.ap())
            projb_sb = pp.tile([128, KC], F32)          # bias for j-tile m in col m
            nc.sync.dma_start(projb_sb[:], projb.ap())
            id64 = pp.tile([64, 64], F32)
            nc.sync.dma_start(id64[:], ident64.ap())
            erep_sb = pp.tile([16, 128], F32)
            nc.sync.dma_start(erep_sb[:], erep.ap())

            # ---- x: column-chunked f32 loads; per-chunk DVE block sums +
            #      ACT cast -> xbf, interleaved with QKV (emitted below) ----
            xbf = pp.tile([128, KC, N], BF16)
            xm = pp.tile([128, KC, NB], F32)

            def emit_xchunk(xp, nch):
                lo, hi = nch * 512, (nch + 1) * 512
                for kc in range(KC):
                    xf = xp.tile([128, 512], F32, tag="xf", name=f"xf_{kc}_{nch}")
                    nc.sync.dma_start(xf[:], xT.ap()[kc * 128:(kc + 1) * 128, lo:hi])
                    nc.vector.tensor_reduce(
                        xm[:, kc, nch * 4:(nch + 1) * 4],
                        xf[:].rearrange("p (b t) -> p b t", t=BLK),
                        axis=mybir.AxisListType.X, op=mybir.AluOpType.add)
                    nc.scalar.copy(xbf[:, kc, lo:hi], xf[:])

            # ---- QKV (bf16) ----
            qT = pp.tile([128, N], BF16)
            kT = pp.tile([128, NB, BLK], BF16)   # contiguous == [128, N]
            v0 = pp.tile([128, NB, 66], BF16)
            v1 = pp.tile([128, NB, 66], BF16)
            nc.vector.memset(v0[:, :, 64:66], 0.0)
            nc.vector.memset(v1[:, :, 64:66], 0.0)
            nc.vector.memset(v0[:, :, 64:65], 1.0)
            nc.vector.memset(v1[:, :, 64:65], 1.0)

            with tc.tile_pool(name="xload", bufs=10) as xp, \
                 tc.tile_pool(name="qkps", bufs=3, space="PSUM") as qp, \
                 tc.tile_pool(name="vps", bufs=3, space="PSUM") as vp:
                for nch in range(8):
                    emit_xchunk(xp, nch)
                    for mt in (0, 1):
                        ps = qp.tile([128, 512], F32, tag="qk")
                        for kc in range(KC):
                            nc.tensor.matmul(
                                ps[:], lhsT=wqkv_bf[:, kc, mt * 128:(mt + 1) * 128],
                                rhs=xbf[:, kc, nch * 512:(nch + 1) * 512],
                                start=(kc == 0), stop=(kc == KC - 1))
                        if mt == 0:
                            nc.scalar.copy(qT[:, nch * 512:(nch + 1) * 512], ps[:])
                        else:
                            nc.scalar.copy(
                                kT[:].rearrange("p a b -> p (a b)")[:, nch * 512:(nch + 1) * 512],
                                ps[:])
                    for nt in range(4 * nch, 4 * nch + 4):
                        psv = vp.tile([128, 128], F32, tag="v")
                        for kc in range(KC):
                            nc.tensor.matmul(psv[:], lhsT=xbf[:, kc, nt * 128:(nt + 1) * 128],
                                             rhs=wqkv_bf[:, kc, 256:384],
                                             start=(kc == 0), stop=(kc == KC - 1))
                        nc.vector.tensor_copy(v0[:, nt, 0:64], psv[:, 0:64])
                        nc.vector.tensor_copy(v1[:, nt, 0:64], psv[:, 64:128])

            # ---- block-mean similarity + top-k selection (f32) ----
            kidx = pp.tile([128, NB], I16)
            vidx0 = pp.tile([128, NB], I16)
            vidx1 = pp.tile([128, NB], I16)
            with tc.tile_pool(name="selps", bufs=2, space="PSUM") as sp, \
                 tc.tile_pool(name="selsb", bufs=2) as sb:
                qm_ps = sp.tile([128, NB], F32, tag="qkm")
                km_ps = sp.tile([128, NB], F32, tag="qkm")
                for kc in range(KC):
                    nc.tensor.matmul(qm_ps[:], lhsT=wqk_f32[:, kc, 0:128],
                                     rhs=xm[:, kc, :], start=(kc == 0), stop=(kc == KC - 1))
                for kc in range(KC):
                    nc.tensor.matmul(km_ps[:], lhsT=wqk_f32[:, kc, 128:256],
                                     rhs=xm[:, kc, :], start=(kc == 0), stop=(kc == KC - 1))
                qm_sb = sb.tile([128, NB], F32, tag="qm")
                km_sb = sb.tile([128, NB], F32, tag="km")
                nc.scalar.copy(qm_sb[:], qm_ps[:])
                nc.scalar.copy(km_sb[:], km_ps[:])

                sim_ps = sp.tile([64, NB], F32, tag="sim")
                for h in range(HPC):
                    nc.tensor.matmul(sim_ps[h * 32:(h + 1) * 32, :],
                                     lhsT=qm_sb[h * 64:(h + 1) * 64, :],
                                     rhs=km_sb[h * 64:(h + 1) * 64, :],
                                     start=True, stop=True)
                sim2 = sb.tile([64, NB], F32, tag="sim2")
                nc.vector.tensor_copy(sim2[:], sim_ps[:])

                vals0 = sb.tile([64, 8], F32, tag="v0")
                idx0 = sb.tile([64, 8], U32, tag="i0")
                pun = sb.tile([64, NB], F32, tag="pun")
                vals1 = sb.tile([64, 8], F32, tag="v1")
                idx1 = sb.tile([64, 8], U32, tag="i1")
                nc.vector.max(vals0[:], sim2[:])
                nc.vector.max_index(idx0[:], vals0[:], sim2[:])
                nc.vector.match_replace(out=pun[:], in_to_replace=vals0[:],
                                        in_values=sim2[:], imm_value=-1e30)
                nc.vector.max(vals1[:], pun[:])
                nc.vector.max_index(idx1[:], vals1[:], pun[:])

                idxf = sb.tile([64, TOPK], F32, tag="idxf")
                nc.vector.tensor_copy(idxf[:, 0:8], idx0[:])
                nc.vector.tensor_copy(idxf[:, 8:16], idx1[:])

                selT_ps = sp.tile([TOPK, 64], F32, tag="selT")
                nc.tensor.transpose(selT_ps[:], idxf[:], id64[:])
                selT = sb.tile([TOPK, 64], F32, tag="selTsb")
                nc.vector.tensor_copy(selT[:], selT_ps[:])

                # replicate selT rows to all 16-partition groups via one matmul:
                # rep[m, n] = selT[m % 16, n]
                rep_ps = sp.tile([128, 64], F32, tag="rep")
                nc.tensor.matmul(rep_ps[:], lhsT=erep_sb[:], rhs=selT[:],
                                 start=True, stop=True)
                nc.vector.tensor_copy(kidx[0:64, :], rep_ps[0:64, 0:32])
                nc.vector.tensor_copy(kidx[64:128, :], rep_ps[64:128, 32:64])
                nc.vector.tensor_copy(vidx0[:], rep_ps[:, 0:32])
                nc.vector.tensor_copy(vidx1[:], rep_ps[:, 32:64])

            # ---- main loop: sparse attention + chunked projection partials ----
            CHQ = 8                    # query blocks per projection chunk
            CHT = CHQ * BLK            # 512 tokens per chunk
            with tc.tile_pool(name="gather", bufs=6) as gp, \
                 tc.tile_pool(name="escore", bufs=14) as ep, \
                 tc.tile_pool(name="sps", bufs=3, space="PSUM") as spp, \
                 tc.tile_pool(name="ops", bufs=2, space="PSUM") as opp, \
                 tc.tile_pool(name="otp", bufs=2) as otp, \
                 tc.tile_pool(name="prout", bufs=4) as pr, \
                 tc.tile_pool(name="osb", bufs=4) as ob:

                def _emit_proj(c):
                    ot = otp.tile([128, CHT], BF16, tag="ot", name=f"ot_{c}")
                    nc.sync.dma_start_transpose(
                        ot[:], obounce.ap()[c * CHT:(c + 1) * CHT, :])
                    for m in range(KC):
                        pj = spp.tile([128, 1024], F32, tag="s", name=f"pj_{c}_{m}")
                        for s2 in range(CHT // 512):
                            nc.tensor.matmul(pj[:, s2 * 512:(s2 + 1) * 512],
                                             lhsT=projW_bf[:, m * 128:(m + 1) * 128],
                                             rhs=ot[:, s2 * 512:(s2 + 1) * 512],
                                             start=True, stop=True)
                        po = pr.tile([128, CHT], F32, tag="po", name=f"po_{c}_{m}")
                        nc.vector.tensor_scalar(po[:], pj[:, 0:CHT],
                                                projb_sb[:, m:m + 1], None,
                                                op0=mybir.AluOpType.add)
                        nc.sync.dma_start(
                            out_ext.ap()[m * 128:(m + 1) * 128, c * CHT:(c + 1) * CHT],
                            po[:])

                state = {}

                def emit_scores(qb):
                    kg = gp.tile([128, TOPK, BLK], BF16, tag="kg",
                                 name=f"kg_{qb}")
                    nc.gpsimd.ap_gather(kg[:], kT[:], kidx[:, qb:qb + 1],
                                        channels=128, num_elems=NB, d=BLK, num_idxs=TOPK)
                    vg0 = gp.tile([128, TOPK, 66], BF16, tag="vg0",
                                  name=f"vg0_{qb}")
                    nc.gpsimd.ap_gather(vg0[:], v0[:], vidx0[:, qb:qb + 1],
                                        channels=128, num_elems=NB, d=66, num_idxs=TOPK)
                    vg1 = gp.tile([128, TOPK, 66], BF16, tag="vg1",
                                  name=f"vg1_{qb}")
                    nc.gpsimd.ap_gather(vg1[:], v1[:], vidx1[:, qb:qb + 1],
                                        channels=128, num_elems=NB, d=66, num_idxs=TOPK)

                    qcol = slice(qb * BLK, (qb + 1) * BLK)
                    etiles = [[None, None], [None, None]]
                    for half in range(2):
                        s0 = spp.tile([128, 1024], F32, tag="s", name=f"s0_{qb}_{half}")
                        s1 = spp.tile([128, 1024], F32, tag="s", name=f"s1_{qb}_{half}")
                        for jj in range(8):
                            j = half * 8 + jj
                            nc.tensor.matmul(s0[:, jj * 128:(jj + 1) * 128],
                                             lhsT=kg[0:64, j, :], rhs=qT[0:64, qcol],
                                             start=True, stop=True)
                            nc.tensor.matmul(s1[:, jj * 128:(jj + 1) * 128],
                                             lhsT=kg[64:128, j, :], rhs=qT[64:128, qcol],
                                             start=True, stop=True)
                        e0 = ep.tile([128, 1024], BF16, tag="e", name=f"e0_{qb}_{half}")
                        e1 = ep.tile([128, 1024], BF16, tag="e", name=f"e1_{qb}_{half}")
                        nc.scalar.activation(e0[:], s0[:],
                                             mybir.ActivationFunctionType.Exp, scale=SCALE)
                        nc.scalar.activation(e1[:], s1[:],
                                             mybir.ActivationFunctionType.Exp, scale=SCALE)
                        etiles[0][half] = e0
                        etiles[1][half] = e1
                    onorm = ob.tile([128, 2 * D], BF16, tag="onorm",
                                    name=f"on_{qb}")
                    state[qb] = (etiles, vg0, vg1, onorm)

                def emit_o(qb, heads=(0, 1)):
                    if qb not in state:
                        return
                    etiles, vg0, vg1, onorm = state[qb]
                    for h in heads:
                        vg = vg0 if h == 0 else vg1
                        o_ps = opp.tile([128, D + 1], F32, tag="o",
                                        name=f"o_{qb}_{h}")
                        for j in range(TOPK):
                            nc.tensor.matmul(o_ps[:],
                                             lhsT=etiles[h][j // 8][:, (j % 8) * 128:(j % 8 + 1) * 128],
                                             rhs=vg[:, j, 0:D + 1],
                                             start=(j == 0), stop=(j == TOPK - 1))
                        rec = ob.tile([128, 1], F32, tag="rec", name=f"r_{qb}_{h}")
                        nc.vector.reciprocal(rec[:], o_ps[:, D:D + 1])
                        nc.vector.tensor_scalar(onorm[:, h * D:(h + 1) * D],
                                                o_ps[:, 0:D], rec[:], None,
                                                op0=mybir.AluOpType.mult)
                    if heads[-1] == 1:
                        state.pop(qb)
                        nc.sync.dma_start(obounce.ap()[qb * BLK:(qb + 1) * BLK, :],
                                          onorm[:])
                        if qb % CHQ == CHQ - 1:
                            _emit_proj(qb // CHQ)

                # software pipeline: o-phase trails scores by one iteration so
                # the exp latency of qb never blocks the PE stream
                for qb in range(NB):
                    emit_scores(qb)
                    emit_o(qb - 1, heads=(0,))
                    emit_o(qb - 1, heads=(1,))
                emit_o(NB - 1, heads=(0,))
                emit_o(NB - 1, heads=(1,))

    nc.compile()
    return nc


def _prep_inputs(x, qkv_w, proj_w, proj_b):
    x = np.asarray(x, dtype=np.float32)
    qkv_w = np.asarray(qkv_w, dtype=np.float32)
    proj_w = np.asarray(proj_w, dtype=np.float32)
    proj_b = np.asarray(proj_b, dtype=np.float32)

    xT = np.ascontiguousarray(x[0].T)                      # [C, N]
    ident64 = np.eye(64, dtype=np.float32)
    erep = (np.arange(128)[None, :] % 16 == np.arange(16)[:, None]).astype(np.float32)
    zero_b = np.zeros((128, 8), dtype=np.float32)
    in_maps = []
    for i in range(NCORES):
        h0 = HPC * i
        rows = []
        for part in range(3):                              # q, k, v row groups
            base = part * C + h0 * D
            rows.append(qkv_w[base:base + HPC * D, :])
        wqkv = np.concatenate(rows, axis=0)                # [384, C]
        cslice = slice(i * 2 * D, (i + 1) * 2 * D)
        in_maps.append({
            "xT": xT,
            "wqkvT": np.ascontiguousarray(wqkv.T),         # [C, 384]
            # [c_local, j]: rows = this core's 128 c-dims, cols = all 1024 j
            "projWT": np.ascontiguousarray(proj_w[:, cslice].T),
            # bias only on core 0 (partials are summed on the host)
            "projb": (np.ascontiguousarray(proj_b.reshape(8, 128).T)
                      if i == 0 else zero_b),
            "ident64": ident64,
            "erep": erep,
        })
    return in_maps


def kernel(x, qkv_w, proj_w, proj_b, _trace=False):
    if "nc" not in _CACHE:
        _CACHE["nc"] = _build()
    nc = _CACHE["nc"]
    in_maps = _prep_inputs(x, qkv_w, proj_w, proj_b)
    res = run_bass_kernel_spmd(nc, in_maps, core_ids=list(range(NCORES)),
                               trace=_trace)
    outT = res.results[0]["out"].astype(np.float32)
    for i in range(1, NCORES):
        outT += res.results[i]["out"]
    out = np.ascontiguousarray(outT.T).reshape(1, N, C).astype(np.float32)
    if _trace:
        _CACHE["last_exec_time_ns"] = res.exec_time_ns
        _CACHE["last_results"] = res
    return out


# revision 30
# speedup vs baseline: 1.0073x; 1.0073x over previous
"""Block-sparse attention (SageAttention-style mean-similarity top-k) on 8 TRN2 NeuronCores.

Sharding: 16 heads tensor-parallel across 8 cores (2 heads/core).
  - qkv weight column-sharded per core (its 2 heads' q/k/v rows, pre-transposed on host)
  - block selection + block-sparse attention fully local per head
  - proj weight row-sharded: each core computes the full-shape PARTIAL product
    o_local @ projW[:, c_slice].T (+ bias on core 0 only); the host unshard step
    sums the 8 partials (the row-parallel reduction).

Per-core device pipeline (bf16 compute, f32 selection):
  x^T f32 -> block sums (DVE) -> qm/km/sim (f32 PE) -> top-16 via max8/max_index
  qkv matmuls (bf16 PE), k kept d-major, v token-major with a ones column
  per query block: ap_gather (GPSIMD ucode, SBUF->SBUF) pulls the 16 selected
  k/v blocks; scores s^T = k_sel^T q (two heads packed in the 128x128 PE array
  via row groups); exp on ACT straight from PSUM; o = (e^T)^T v_sel with the
  gathered ones column yielding the softmax denominator; per-partition
  normalize; chunk-wise DMA-transpose + projection partials streamed out.
"""

import os
import sys

for _p in ("/opt/trn_rl_repo", "/root/.axon_site/_ro/trn_rl_repo"):
    if os.path.isdir(_p) and _p not in sys.path:
        sys.path.insert(0, _p)

import numpy as np

import concourse.bass as bass
import concourse.bacc as bacc
import concourse.tile as tile
import concourse.mybir as mybir
from concourse.bass_utils import run_bass_kernel_spmd
from concourse.library_config import ap_gather as ap_gather_lib

# problem constants
N = 4096          # sequence length
C = 1024          # model dim
H = 16            # heads
D = 64            # head dim
BLK = 128         # block size
NB = N // BLK     # 32 blocks
TOPK = 16         # int(0.5 * NB)
NCORES = 8
HPC = H // NCORES  # 2 heads per core
SCALE = D ** -0.5  # 0.125

F32 = mybir.dt.float32
BF16 = mybir.dt.bfloat16
I16 = mybir.dt.int16
U32 = mybir.dt.uint32

_CACHE = {}


def _build():
    nc = bacc.Bacc("TRN2", target_bir_lowering=False, debug=False,
                   num_devices=NCORES)

    KC = C // 128  # 8 contraction tiles

    xT = nc.dram_tensor("xT", [C, N], F32, kind="ExternalInput")
    wqkvT = nc.dram_tensor("wqkvT", [C, 3 * 2 * D], F32, kind="ExternalInput")
    projWT = nc.dram_tensor("projWT", [2 * D, C], F32, kind="ExternalInput")
    projb = nc.dram_tensor("projb", [128, KC], F32, kind="ExternalInput")
    ident64 = nc.dram_tensor("ident64", [64, 64], F32, kind="ExternalInput")
    erep = nc.dram_tensor("erep", [16, 128], F32, kind="ExternalInput")
    out_ext = nc.dram_tensor("out", [C, N], F32, kind="ExternalOutput")

    obounce = nc.dram_tensor("obounce", [N, 2 * D], BF16)

    with tile.TileContext(nc) as tc:
        nc.gpsimd.load_library(ap_gather_lib)

        with tc.tile_pool(name="persist", bufs=1) as pp:
            # ---- weights ----
            wqkv_bf = pp.tile([128, KC, 384], BF16)
            wqk_f32 = pp.tile([128, KC, 384], F32)
            nc.sync.dma_start(
                wqk_f32[:], wqkvT.ap().rearrange("(a p) m -> p a m", p=128))
            nc.scalar.copy(wqkv_bf[:], wqk_f32[:])
            projW_bf = pp.tile([128, C], BF16)          # [c_local, j]
            nc.gpsimd.dma_start(projW_bf[:], projWT.ap())
            projb_sb = pp.tile([128, KC], F32)          # bias for j-tile m in col m
            nc.sync.dma_start(projb_sb[:], projb.ap())
            id64 = pp.tile([64, 64], F32)
            nc.sync.dma_start(id64[:], ident64.ap())
            erep_sb = pp.tile([16, 128], F32)
            nc.sync.dma_start(erep_sb[:], erep.ap())

            # ---- x: column-chunked f32 loads; per-chunk DVE block sums +
            #      ACT cast -> xbf, interleaved with QKV (emitted below) ----
            xbf = pp.tile([128, KC, N], BF16)
            xm = pp.tile([128, KC, NB], F32)

            def emit_xchunk(xp, nch):
                lo, hi = nch * 512, (nch + 1) * 512
                for kc in range(KC):
                    xf = xp.tile([128, 512], F32, tag="xf", name=f"xf_{kc}_{nch}")
                    nc.sync.dma_start(xf[:], xT.ap()[kc * 128:(kc + 1) * 128, lo:hi])
                    nc.vector.tensor_reduce(
                        xm[:, kc, nch * 4:(nch + 1) * 4],
                        xf[:].rearrange("p (b t) -> p b t", t=BLK),
                        axis=mybir.AxisListType.X, op=mybir.AluOpType.add)
                    nc.scalar.copy(xbf[:, kc, lo:hi], xf[:])

            # ---- QKV (bf16) ----
            qT = pp.tile([128, N], BF16)
            kT = pp.tile([128, NB, BLK], BF16)   # contiguous == [128, N]
            v0 = pp.tile([128, NB, 66], BF16)
            v1 = pp.tile([128, NB, 66], BF16)
            nc.vector.memset(v0[:, :, 64:66], 0.0)
            nc.vector.memset(v1[:, :, 64:66], 0.0)
            nc.vector.memset(v0[:, :, 64:65], 1.0)
            nc.vector.memset(v1[:, :, 64:65], 1.0)

            with tc.tile_pool(name="xload", bufs=10) as xp, \
                 tc.tile_pool(name="qkps", bufs=3, space="PSUM") as qp, \
                 tc.tile_pool(name="vps", bufs=3, space="PSUM") as vp:
                for nch in range(8):
                    emit_xchunk(xp, nch)
                    for mt in (0, 1):
                        ps = qp.tile([128, 512], F32, tag="qk")
                        for kc in range(KC):
                            nc.tensor.matmul(
                                ps[:], lhsT=wqkv_bf[:, kc, mt * 128:(mt + 1) * 128],
                                rhs=xbf[:, kc, nch * 512:(nch + 1) * 512],
                                start=(kc == 0), stop=(kc == KC - 1))
                        if mt == 0:
                            nc.scalar.copy(qT[:, nch * 512:(nch + 1) * 512], ps[:])
                        else:
                            nc.scalar.copy(
                                kT[:].rearrange("p a b -> p (a b)")[:, nch * 512:(nch + 1) * 512],
                                ps[:])
                    for nt in range(4 * nch, 4 * nch + 4):
                        psv = vp.tile([128, 128], F32, tag="v")
                        for kc in range(KC):
                            nc.tensor.matmul(psv[:], lhsT=xbf[:, kc, nt * 128:(nt + 1) * 128],
                                             rhs=wqkv_bf[:, kc, 256:384],
                                             start=(kc == 0), stop=(kc == KC - 1))
                        nc.vector.tensor_copy(v0[:, nt, 0:64], psv[:, 0:64])
                        nc.vector.tensor_copy(v1[:, nt, 0:64], psv[:, 64:128])

            # ---- block-mean similarity + top-k selection (f32) ----
            kidx = pp.tile([128, NB], I16)
            vidx0 = pp.tile([128, NB], I16)
            vidx1 = pp.tile([128, NB], I16)
            with tc.tile_pool(name="selps", bufs=2, space="PSUM") as sp, \
                 tc.tile_pool(name="selsb", bufs=2) as sb:
                qm_ps = sp.tile([128, NB], F32, tag="qkm")
                km_ps = sp.tile([128, NB], F32, tag="qkm")
                for kc in range(KC):
                    nc.tensor.matmul(qm_ps[:], lhsT=wqk_f32[:, kc, 0:128],
                                     rhs=xm[:, kc, :], start=(kc == 0), stop=(kc == KC - 1))
                for kc in range(KC):
                    nc.tensor.matmul(km_ps[:], lhsT=wqk_f32[:, kc, 128:256],
                                     rhs=xm[:, kc, :], start=(kc == 0), stop=(kc == KC - 1))
                qm_sb = sb.tile([128, NB], F32, tag="qm")
                km_sb = sb.tile([128, NB], F32, tag="km")
                nc.scalar.copy(qm_sb[:], qm_ps[:])
                nc.scalar.copy(km_sb[:], km_ps[:])

                sim_ps = sp.tile([64, NB], F32, tag="sim")
                for h in range(HPC):
                    nc.tensor.matmul(sim_ps[h * 32:(h + 1) * 32, :],
                                     lhsT=qm_sb[h * 64:(h + 1) * 64, :],
                                     rhs=km_sb[h * 64:(h + 1) * 64, :],
                                     start=True, stop=True)
                sim2 = sb.tile([64, NB], F32, tag="sim2")
                nc.vector.tensor_copy(sim2[:], sim_ps[:])

                vals0 = sb.tile([64, 8], F32, tag="v0")
                idx0 = sb.tile([64, 8], U32, tag="i0")
                pun = sb.tile([64, NB], F32, tag="pun")
                vals1 = sb.tile([64, 8], F32, tag="v1")
                idx1 = sb.tile([64, 8], U32, tag="i1")
                nc.vector.max(vals0[:], sim2[:])
                nc.vector.max_index(idx0[:], vals0[:], sim2[:])
                nc.vector.match_replace(out=pun[:], in_to_replace=vals0[:],
                                        in_values=sim2[:], imm_value=-1e30)
                nc.vector.max(vals1[:], pun[:])
                nc.vector.max_index(idx1[:], vals1[:], pun[:])

                idxf = sb.tile([64, TOPK], F32, tag="idxf")
                nc.vector.tensor_copy(idxf[:, 0:8], idx0[:])
                nc.vector.tensor_copy(idxf[:, 8:16], idx1[:])

                selT_ps = sp.tile([TOPK, 64], F32, tag="selT")
                nc.tensor.transpose(selT_ps[:], idxf[:], id64[:])
                selT = sb.tile([TOPK, 64], F32, tag="selTsb")
                nc.vector.tensor_copy(selT[:], selT_ps[:])

                # replicate selT rows to all 16-partition groups via one matmul:
                # rep[m, n] = selT[m % 16, n]
                rep_ps = sp.tile([128, 64], F32, tag="rep")
                nc.tensor.matmul(rep_ps[:], lhsT=erep_sb[:], rhs=selT[:],
                                 start=True, stop=True)
                nc.vector.tensor_copy(kidx[0:64, :], rep_ps[0:64, 0:32])
                nc.vector.tensor_copy(kidx[64:128, :], rep_ps[64:128, 32:64])
                nc.vector.tensor_copy(vidx0[:], rep_ps[:, 0:32])
                nc.vector.tensor_copy(vidx1[:], rep_ps[:, 32:64])

            # ---- main loop: sparse attention + chunked projection partials ----
            CHQ = 8                    # query blocks per projection chunk
            CHT = CHQ * BLK            # 512 tokens per chunk
            with tc.tile_pool(name="gather", bufs=6) as gp, \
                 tc.tile_pool(name="escore", bufs=12) as ep, \
                 tc.tile_pool(name="sps", bufs=3, space="PSUM") as spp, \
                 tc.tile_pool(name="ops", bufs=2, space="PSUM") as opp, \
                 tc.tile_pool(name="otp", bufs=2) as otp, \
                 tc.tile_pool(name="prout", bufs=4) as pr, \
                 tc.tile_pool(name="osb", bufs=4) as ob:

                def _emit_proj(c):
                    ot = otp.tile([128, CHT], BF16, tag="ot", name=f"ot_{c}")
                    nc.sync.dma_start_transpose(
                        ot[:], obounce.ap()[c * CHT:(c + 1) * CHT, :])
                    for m in range(KC):
                        pj = spp.tile([128, 1024], F32, tag="s", name=f"pj_{c}_{m}")
                        for s2 in range(CHT // 512):
                            nc.tensor.matmul(pj[:, s2 * 512:(s2 + 1) * 512],
                                             lhsT=projW_bf[:, m * 128:(m + 1) * 128],
                                             rhs=ot[:, s2 * 512:(s2 + 1) * 512],
                                             start=True, stop=True)
                        po = pr.tile([128, CHT], F32, tag="po", name=f"po_{c}_{m}")
                        nc.vector.tensor_scalar(po[:], pj[:, 0:CHT],
                                                projb_sb[:, m:m + 1], None,
                                                op0=mybir.AluOpType.add)
                        nc.sync.dma_start(
                            out_ext.ap()[m * 128:(m + 1) * 128, c * CHT:(c + 1) * CHT],
                            po[:])

                state = {}

                def emit_scores(qb):
                    kg = gp.tile([128, TOPK, BLK], BF16, tag="kg",
                                 name=f"kg_{qb}")
                    nc.gpsimd.ap_gather(kg[:], kT[:], kidx[:, qb:qb + 1],
                                        channels=128, num_elems=NB, d=BLK, num_idxs=TOPK)
                    vg0 = gp.tile([128, TOPK, 66], BF16, tag="vg0",
                                  name=f"vg0_{qb}")
                    nc.gpsimd.ap_gather(vg0[:], v0[:], vidx0[:, qb:qb + 1],
                                        channels=128, num_elems=NB, d=66, num_idxs=TOPK)
                    vg1 = gp.tile([128, TOPK, 66], BF16, tag="vg1",
                                  name=f"vg1_{qb}")
                    nc.gpsimd.ap_gather(vg1[:], v1[:], vidx1[:, qb:qb + 1],
                                        channels=128, num_elems=NB, d=66, num_idxs=TOPK)

                    qcol = slice(qb * BLK, (qb + 1) * BLK)
                    etiles = [[None, None], [None, None]]
                    for half in range(2):
                        s0 = spp.tile([128, 1024], F32, tag="s", name=f"s0_{qb}_{half}")
                        s1 = spp.tile([128, 1024], F32, tag="s", name=f"s1_{qb}_{half}")
                        for jj in range(8):
                            j = half * 8 + jj
                            nc.tensor.matmul(s0[:, jj * 128:(jj + 1) * 128],
                                             lhsT=kg[0:64, j, :], rhs=qT[0:64, qcol],
                                             start=True, stop=True)
                            nc.tensor.matmul(s1[:, jj * 128:(jj + 1) * 128],
                                             lhsT=kg[64:128, j, :], rhs=qT[64:128, qcol],
                                             start=True, stop=True)
                        e0 = ep.tile([128, 1024], BF16, tag="e", name=f"e0_{qb}_{half}")
                        e1 = ep.tile([128, 1024], BF16, tag="e", name=f"e1_{qb}_{half}")
                        nc.scalar.activation(e0[:], s0[:],
                                             mybir.ActivationFunctionType.Exp, scale=SCALE)
                        nc.scalar.activation(e1[:], s1[:],
                                             mybir.ActivationFunctionType.Exp, scale=SCALE)
                        etiles[0][half] = e0
                        etiles[1][half] = e1
                    onorm = ob.tile([128, 2 * D], BF16, tag="onorm",
                                    name=f"on_{qb}")
                    state[qb] = (etiles, vg0, vg1, onorm)

                def emit_o(qb, heads=(0, 1)):
                    if qb not in state:
                        return
                    etiles, vg0, vg1, onorm = state[qb]
                    for h in heads:
                        vg = vg0 if h == 0 else vg1
                        o_ps = opp.tile([128, D + 1], F32, tag="o",
                                        name=f"o_{qb}_{h}")
                        for j in range(TOPK):
                            nc.tensor.matmul(o_ps[:],
                                             lhsT=etiles[h][j // 8][:, (j % 8) * 128:(j % 8 + 1) * 128],
                                             rhs=vg[:, j, 0:D + 1],
                                             start=(j == 0), stop=(j == TOPK - 1))
                        rec = ob.tile([128, 1], F32, tag="rec", name=f"r_{qb}_{h}")
                        nc.vector.reciprocal(rec[:], o_ps[:, D:D + 1])
                        nc.vector.tensor_scalar(onorm[:, h * D:(h + 1) * D],
                                                o_ps[:, 0:D], rec[:], None,
                                                op0=mybir.AluOpType.mult)
                    if heads[-1] == 1:
                        state.pop(qb)
                        nc.sync.dma_start(obounce.ap()[qb * BLK:(qb + 1) * BLK, :],
                                          onorm[:])
                        if qb % CHQ == CHQ - 1:
                            _emit_proj(qb // CHQ)

                # software pipeline: o-phase trails scores by one iteration so
                # the exp latency of qb never blocks the PE stream
                for qb in range(NB):
                    emit_scores(qb)
                    emit_o(qb - 1, heads=(0,))
                    emit_o(qb - 1, heads=(1,))
                emit_o(NB - 1, heads=(0,))
                emit_o(NB - 1, heads=(1,))

    nc.compile()
    return nc


def _prep_inputs(x, qkv_w, proj_w, proj_b):
    x = np.asarray(x, dtype=np.float32)
    qkv_w = np.asarray(qkv_w, dtype=np.float32)
    proj_w = np.asarray(proj_w, dtype=np.float32)
    proj_b = np.asarray(proj_b, dtype=np.float32)

    xT = np.ascontiguousarray(x[0].T)                      # [C, N]
    ident64 = np.eye(64, dtype=np.float32)
    erep = (np.arange(128)[None, :] % 16 == np.arange(16)[:, None]).astype(np.float32)
    zero_b = np.zeros((128, 8), dtype=np.float32)
    in_maps = []
    for i in range(NCORES):
        h0 = HPC * i
        rows = []
        for part in range(3):                              # q, k, v row groups
            base = part * C + h0 * D
            rows.append(qkv_w[base:base + HPC * D, :])
        wqkv = np.concatenate(rows, axis=0)                # [384, C]
        cslice = slice(i * 2 * D, (i + 1) * 2 * D)
        in_maps.append({
            "xT": xT,
            "wqkvT": np.ascontiguousarray(wqkv.T),         # [C, 384]
            # [c_local, j]: rows = this core's 128 c-dims, cols = all 1024 j
            "projWT": np.ascontiguousarray(proj_w[:, cslice].T),
            # bias only on core 0 (partials are summed on the host)
            "projb": (np.ascontiguousarray(proj_b.reshape(8, 128).T)
                      if i == 0 else zero_b),
            "ident64": ident64,
            "erep": erep,
        })
    return in_maps


def kernel(x, qkv_w, proj_w, proj_b, _trace=False):
    if "nc" not in _CACHE:
        _CACHE["nc"] = _build()
    nc = _CACHE["nc"]
    in_maps = _prep_inputs(x, qkv_w, proj_w, proj_b)
    res = run_bass_kernel_spmd(nc, in_maps, core_ids=list(range(NCORES)),
                               trace=_trace)
    outT = res.results[0]["out"].astype(np.float32)
    for i in range(1, NCORES):
        outT += res.results[i]["out"]
    out = np.ascontiguousarray(outT.T).reshape(1, N, C).astype(np.float32)
    if _trace:
        _CACHE["last_exec_time_ns"] = res.exec_time_ns
        _CACHE["last_results"] = res
    return out
